# revision 40
# baseline (speedup 1.0000x reference)
"""Trainium2 Bass kernel for CausalModulatedAttention (transposed form).

Full-input contract: kernel(**inputs) takes the unsharded numpy inputs and
returns the full (B, L, D) float32 output.

Sharding: core = 2*b + g (b = batch, g = head-group of 8 heads).  Everything
is computed in j-transposed orientation so softmax-ready tiles feed attn@v
directly as matmul moving operands (no PE transposes / PSUM round trips):

  - scores^T[j, i] = k_j . q_i on PE (j chunked into 4 blocks of 128);
    exp(scores) on ACT runs independently of the causal-graph bias.
  - the pairwise causal graph (shared by all heads) is rank-split in
    interleaved 4-row j-groups (j = 8k + 4g + u) so both cores of a pair do
    identical-shape, balanced causal work: gelu(hc_i + he_j + b1) over the
    causal suffix (the hc_i + he_j adds split between DVE and GpSimd),
    reduced over the hidden dim by sparse 32-col stationary matmuls, then
    tanh.  Halves are exchanged as fp8 through one per-pair AllGather and
    re-interleaved by two constant fp8 permutation matmuls per j-chunk; the
    bias enters multiplicatively as e2 = exp(qk) * expG with
    expG = exp(alpha/2*tanh + trimask) (mask -> exp = 0).
  - a zero-dependency warmup AllGather triggers at ~t0 (its 4KB input DMA
    is the first transfer on the sync ring) so the global entry barrier +
    ncfw bootstrap complete while inputs stream in, and the real exchange
    starts the moment its trigger fires.
  - softmax row sums come from ones-column matmuls, col-tiled 4 ways
    (head-pair hp -> psum partitions 32hp+{0,4}) so the 4 chains run
    concurrently in different PE quadrants; 1/x runs on ACT (table
    preloaded at t0), broadcast back to head-layout by a constant matmul.
  - attn@v: otp[hd, i] accumulates over j-chunks with suffix extents.
  - each core emits a partial output (its heads' half of the d-contraction
    in the final projection); the host adds the two halves.

All matmul operands bf16 (exchange/permute fp8), fp32 PSUM accumulation.
"""

import math

import numpy as np
import ml_dtypes

import concourse.bass as bass
import concourse.mybir as mybir
import concourse.tile as tile
from concourse import bacc
from concourse.bass_utils import run_bass_kernel_spmd

BF = mybir.dt.bfloat16
F8 = mybir.dt.float8e4
F32 = mybir.dt.float32
AF = mybir.ActivationFunctionType
ALU = mybir.AluOpType

B, L, D = 4, 512, 1024
H, HD, CD = 16, 64, 32
ALPHA = 0.3
N_CORES = 8
HPC = 8               # heads per core
DPC = HPC * HD        # 512 d-columns per core
NEG = -1.0e30
AW, BW = 512, 256     # i-extents of pairwise blocks A (k 0..31) and B (32..63)
XW = AW + BW          # exchanged columns (A then B)
# cpk packing (fp8): w2t (1024) | ones8 (64) | sel8 (512); trimask ships
# separately in bf16 (DVE reads it); p8 carries P0|P1 as fp8 for the gx
# permutes
CPK8 = 1024 + 64 + 512


def _bf(a):
    return np.ascontiguousarray(a.astype(ml_dtypes.bfloat16))


def _f8(a):
    return np.ascontiguousarray(np.asarray(a).astype(ml_dtypes.float8_e4m3))


def _f32(a):
    return np.ascontiguousarray(a.astype(np.float32))


def build_program():
    nc = bacc.Bacc("TRN2", num_devices=N_CORES, target_bir_lowering=False,
                   debug=False)

    boot_d = nc.dram_tensor("boot", [128, 1280], F8, kind="ExternalInput")
    xta_d = nc.dram_tensor("xta", [128, 8 * L], BF, kind="ExternalInput")
    xje_d = nc.dram_tensor("xje", [128, 8 * 256], F8, kind="ExternalInput")
    cpk_d = nc.dram_tensor("cpk", [128, CPK8], F8, kind="ExternalInput")
    tri_d = nc.dram_tensor("tri", [128, 512], BF, kind="ExternalInput")
    p8_d = nc.dram_tensor("p8", [128, 256], F8, kind="ExternalInput")
    wk_d = nc.dram_tensor("wka", [128, 8 * DPC], BF, kind="ExternalInput")
    wq_d = nc.dram_tensor("wqa", [128, 8 * DPC], BF, kind="ExternalInput")
    wv_d = nc.dram_tensor("wva", [128, 8 * DPC], BF, kind="ExternalInput")
    wo_d = nc.dram_tensor("woa", [128, 4 * D], BF, kind="ExternalInput")
    b1_d = nc.dram_tensor("b1x4", [128, 1], F32, kind="ExternalInput")
    b2_d = nc.dram_tensor("b2h", [128, 1], F32, kind="ExternalInput")
    out_d = nc.dram_tensor("out", [L, D], BF, kind="ExternalOutput")

    with tile.TileContext(nc) as tc:
        with (
            tc.tile_pool(name="consts", bufs=1) as consts,
            tc.tile_pool(name="work", bufs=2) as work,
            tc.tile_pool(name="etp", bufs=8) as etp,
            tc.tile_pool(name="dram", bufs=1, space="DRAM") as dpool,
            tc.tile_pool(name="pps", bufs=2, space="PSUM") as pps,
        ):
            # ---------- warmup collective: zero-dependency, triggers ~t0 ---
            # emitted first + high priority so the bass collectives-bootstrap
            # barrier DMAs and the tiny di transfer land at the HEAD of
            # their rings, before any bulk input load.
            di = dpool.tile([128, 16], BF, tag="di")
            do = dpool.tile([2, 128, 16], BF, tag="do")
            dsb = consts.tile([128, 16], BF, tag="dsb")
            bootp = consts.tile([128, 1280], F8, tag="bootp")
            cpk = consts.tile([128, CPK8], F8, tag="cpk")
            trimask = consts.tile([128, 512], BF, tag="tri")
            wva = consts.tile([128, 8 * DPC], BF, tag="wva")
            p8 = consts.tile([128, 256], F8, tag="p8")
            with tc.high_priority():
                # NOTE: di is never written -- the warmup AllGather only
                # exists to run the collectives bootstrap + barrier early,
                # its payload is garbage by design.  No producer means the
                # trigger fires the moment the gpsimd queue reaches it.
                nc.gpsimd.memset(dsb[:], 0.0)
                # gpsimd-ring loads enqueue BEFORE the blocking collective
                # instruction; the ring drains while the queue waits.
                nc.gpsimd.dma_start(out=bootp[:], in_=boot_d[:, :])
                nc.gpsimd.dma_start(out=cpk[:, 0:CPK8], in_=cpk_d[:, 0:CPK8])
                nc.gpsimd.dma_start(out=trimask[:], in_=tri_d[:, :])
                nc.gpsimd.dma_start(out=wva[:, 0:2048], in_=wv_d[:, 0:2048])
                nc.gpsimd.dma_start(out=wva[:, 2048:4096], in_=wv_d[:, 2048:4096])
                nc.gpsimd.dma_start(out=p8[:], in_=p8_d[:, :])
            # warm the custom-DVE reciprocal (param-RAM load) at t0
            rwarm = consts.tile([128, 1], F32, tag="rwarm")
            nc.vector.memset(rwarm[:], 1.0)
            rwo = consts.tile([128, 1], F32, tag="rwo")
            nc.vector.reciprocal_approx_fast(out=rwo[:], in_=rwarm[:])

            # ---------- input DMAs, first-needed-first per ring ------------
            b1x4 = consts.tile([128, 1], F32, tag="b1x4")
            nc.scalar.dma_start(out=b1x4[:], in_=b1_d[:, :])
            b2h = consts.tile([128, 1], F32, tag="b2h")
            nc.scalar.dma_start(out=b2h[:], in_=b2_d[:, :])
            xje = consts.tile([128, 8 * 256], F8, tag="xje")
            nc.scalar.dma_start(out=xje[:], in_=xje_d[:, :])
            # per-m-chunk xta pieces: the hc/hej/proj chains consume chunk
            # mc as soon as it lands instead of waiting for a 512KB block
            xta = consts.tile([128, 8 * L], BF, tag="xta")
            for mc in range(4):
                nc.sync.dma_start(out=xta[:, mc * L:(mc + 1) * L],
                                  in_=xta_d[:, mc * L:(mc + 1) * L])
                nc.scalar.dma_start(out=xta[:, (4 + mc) * L:(5 + mc) * L],
                                    in_=xta_d[:, (4 + mc) * L:(5 + mc) * L])
            # wka/wqa are packed dc-major: chunk so proj_kq(dc) only waits
            # on its own quarter
            wka = consts.tile([128, 8 * DPC], BF, tag="wka")
            wqa = consts.tile([128, 8 * DPC], BF, tag="wqa")
            for dc in range(4):
                nc.scalar.dma_start(out=wka[:, dc * 1024:(dc + 1) * 1024],
                                    in_=wk_d[:, dc * 1024:(dc + 1) * 1024])
                nc.sync.dma_start(out=wqa[:, dc * 1024:(dc + 1) * 1024],
                                  in_=wq_d[:, dc * 1024:(dc + 1) * 1024])
            woa = consts.tile([128, 4 * D], BF, tag="woa")
            nc.sync.dma_start(out=woa[:], in_=wo_d[:, :])

            # warmup AllGather: emitted AFTER the bulk dma_start
            # instructions -- bass's collectives bootstrap (emitted at the
            # first collective) inserts a sync-queue wait on the global
            # 8-core barrier, which would otherwise freeze the sync DMA
            # ring (and every input load on it) until the slowest core
            # arrives.  Here only gin/gx sit behind it.
            nc.gpsimd.collective_compute(
                "AllGather", ALU.bypass,
                replica_groups=[[0, 1], [2, 3], [4, 5], [6, 7]],
                ins=[di[:, :].opt()], outs=[do[:, :, :].opt()])

            boot = bootp
            xT = [xta[:, mc * L:(mc + 1) * L] for mc in range(8)]
            xj = [xje[:, mc * 256:(mc + 1) * 256] for mc in range(8)]
            wc1x4 = boot[:, 0:1024]
            we1ch = boot[:, 1024:1280]
            w2t = cpk[:, 0:1024]
            ones8 = cpk[:, 1024:1088]
            sel8 = cpk[0:8, 1088:1600]
            P0 = p8[:, 0:128]
            P1 = p8[:, 128:256]
            # wka/wqa are dc-major: [dc, mc, 128]
            wv = [wva[:, mc * DPC:(mc + 1) * DPC] for mc in range(8)]
            wo = [woa[:, dc * D:(dc + 1) * D] for dc in range(4)]

            # ---------- hej4 / hcfull4 (unblock the gelu chain) ----------
            # hej4[u*32+c, kk]: kk 0..31 = own A rows (j = 8k+4g+u),
            # kk 32..63 = B rows as-rank-0 (j = 8k+u, k = kk),
            # kk 64..95 = B rows as-rank-1 (j = 8k+4+u, k = kk-32)
            ps = pps.tile([128, 64], F32, tag="ps")
            for u in range(4):
                for mc in range(8):
                    nc.tensor.matmul(ps[u * CD:(u + 1) * CD, :],
                                     we1ch[:, mc * CD:(mc + 1) * CD],
                                     xj[mc][:, u * 64:(u + 1) * 64],
                                     start=(mc == 0), stop=(mc == 7),
                                     tile_position=(0, u * CD))
            hej4 = consts.tile([128, 64], BF, tag="hej4")
            nc.vector.tensor_copy(hej4[:], ps[:])

            # hcfull4[r*32+c, i] = (x @ Wc1)[i, c] + b1[c]   (replicated over r)
            ps = pps.tile([128, L], F32, tag="ps")
            for mc in range(8):
                nc.tensor.matmul(ps[:], wc1x4[:, mc * 128:(mc + 1) * 128],
                                 xT[mc], start=(mc == 0), stop=(mc == 7))
            hcfull4 = consts.tile([128, L], BF, tag="hcfull4")
            nc.vector.tensor_scalar_add(hcfull4[:], ps[:], b1x4[:, 0:1])

            # ---------- pairwise causal-graph bias (transposed, own half) ----
            # gallA: j-groups k 0..31 (i in [0,512)); gallB: k 32..63
            # (i in [256,512)).  fd(k) = 512 - 8k.
            gallAB = consts.tile([128, XW], F8, tag="gallAB")
            gallA = gallAB[:, 0:AW]
            gallB = gallAB[:, AW:XW]

            def pairwise(bb, graw, hoff, i0):
                ks = range(bb * 8, bb * 8 + 8)
                fds = [L - 8 * k for k in ks]
                offs = [sum(fds[:n]) for n in range(8)]
                tot = sum(fds)
                t4 = work.tile([128, 3872], BF, tag="t4", bufs=3)
                for n, k in enumerate(ks):
                    # broadcast hej col: tensor_tensor keeps 2x DVE mode
                    # (the per-partition-scalar form runs 1x)
                    nc.vector.tensor_add(
                        t4[:, offs[n]:offs[n] + fds[n]],
                        hcfull4[:, 8 * k:L],
                        hej4[:, hoff + k:hoff + k + 1]
                        .broadcast_to((128, fds[n])))
                ga = work.tile([128, 3872], BF, tag="ga", bufs=3)
                nc.scalar.activation(ga[:, 0:tot], t4[:, 0:tot], AF.Gelu)
                for n, k in enumerate(ks):
                    t = k % 32
                    gb = 32 * (t // 8)
                    nc.tensor.matmul(graw[gb:gb + 32, 8 * k - i0:],
                                     w2t[:, t * 32:(t + 1) * 32],
                                     ga[:, offs[n]:offs[n] + fds[n]],
                                     start=(t % 8 == 0), stop=(t % 8 == 7),
                                     tile_position=(0, gb))

            def gfin(graw, gall, w):
                nc.scalar.activation(gall[:, 0:w], graw[:, 0:w], AF.Tanh,
                                     scale=0.5, bias=b2h[:, 0:1])

            pgr_ctx = tc.tile_pool(name="pgr", bufs=1, space="PSUM")
            pgr = pgr_ctx.__enter__()
            with tc.high_priority():
                grawA = pgr.tile([128, AW], F32, tag="graw")
                nc.vector.memset(grawA[:], 0.0)
                for bb in range(4):
                    pairwise(bb, grawA, 0, 0)

                # ---------- pairwise B (own half) ----------
                grawB = pgr.tile([128, BW], F32, tag="graw")
                nc.vector.memset(grawB[:], 0.0)
                for bb in range(4, 8):
                    pairwise(bb, grawB, 0, 256)
                # tanhs grouped after all gelus: the ACT engine holds only
                # two function tables -- alternating costs a 1.3us reload
                gfin(grawA, gallA, AW)
                gfin(grawB, gallB, BW)

            # ---------- exchange both halves within the pair ----------
            gin = dpool.tile([128, XW], F8, tag="gin")
            gout = dpool.tile([2, 128, XW], F8, tag="gout")
            with tc.high_priority():
                nc.sync.dma_start(out=gin[:, :], in_=gallAB[:])
                nc.gpsimd.collective_compute(
                    "AllGather", ALU.bypass,
                    replica_groups=[[0, 1], [2, 3], [4, 5], [6, 7]],
                    ins=[gin[:, :].opt()], outs=[gout[:, :, :].opt()])
            gx = []
            for r in range(2):
                t = consts.tile([128, XW], F8, tag=f"gx{r}")
                nc.sync.dma_start(out=t[:], in_=gout[r, :, :])
                gx.append(t)

            # ---------- projection emitters ----------
            kT, qT, v = [None] * 4, [None] * 4, [None] * 4

            def proj_kq(dc):
                ps = pps.tile([128, L], F32, tag="ps")
                for mc in range(8):
                    nc.tensor.matmul(
                        ps[:], wka[:, dc * 1024 + mc * 128:dc * 1024 + (mc + 1) * 128],
                        xT[mc], start=(mc == 0), stop=(mc == 7))
                t = consts.tile([128, L], BF, tag=f"kT{dc}")
                nc.vector.tensor_copy(t[:], ps[:])
                kT[dc] = t
                ps = pps.tile([128, L], F32, tag="ps")
                for mc in range(8):
                    nc.tensor.matmul(
                        ps[:], wqa[:, dc * 1024 + mc * 128:dc * 1024 + (mc + 1) * 128],
                        xT[mc], start=(mc == 0), stop=(mc == 7))
                t = consts.tile([128, L], BF, tag=f"qT{dc}")
                nc.vector.tensor_copy(t[:], ps[:])
                qT[dc] = t

            def proj_v(jc):
                t = consts.tile([128, DPC], BF, tag=f"v{jc}")
                ps = pps.tile([128, DPC], F32, tag="ps")
                for mc in range(8):
                    nc.tensor.matmul(ps[:], xT[mc][:, jc * 128:(jc + 1) * 128],
                                     wv[mc], start=(mc == 0), stop=(mc == 7))
                nc.vector.tensor_copy(t[:], ps[:])
                v[jc] = t

            proj_kq(0)
            proj_kq(1)
            proj_v(2)
            proj_v(3)
            proj_kq(2)
            proj_kq(3)

            # ---------- scores + exp(scores): independent of the exchange --
            attpool = ctx_att = tc.tile_pool(name="att", bufs=1, space="PSUM")
            psc = pot = prs = attpool.__enter__()

            eR = [[None] * 4 for _ in range(4)]

            def scores_exp(hp, jc):
                iext = L - 128 * jc
                sc = psc.tile([128, 1024], F32, tag="sc", bufs=2)
                for sub in range(2):
                    po = 64 * sub
                    nc.tensor.matmul(
                        sc[:, sub * 512:sub * 512 + iext],
                        kT[hp][po:po + 64, jc * 128:(jc + 1) * 128],
                        qT[hp][po:po + 64, jc * 128:L],
                        start=True, stop=True, tile_position=(po, 0))
                e = etp.tile([128, 2 * iext], BF, tag="eRaw", bufs=16)
                src3 = sc[:].rearrange("p (s n) -> p s n", s=2)[:, :, 0:iext]
                dst3 = e[:].rearrange("p (s n) -> p s n", s=2)[:, :, 0:iext]
                nc.scalar.activation(dst3, src3, AF.Exp)
                eR[hp][jc] = e

            # ---------- re-interleave G^T chunks (2 perm matmuls each) ------
            # GTc[jc] covers j in [128jc, 128jc+128), i in [128jc, 512)
            GTc = [None] * 4

            def make_gtc(jc):
                iext = L - 128 * jc
                half = slice(0, 64) if jc % 2 == 0 else slice(64, 128)
                src = ([gx[0][:, 0:AW], gx[1][:, 0:AW]] if jc < 2
                       else [gx[0][:, AW:XW], gx[1][:, AW:XW]])
                cs = slice(128 * (jc % 2), (AW if jc < 2 else BW))
                ps = pps.tile([128, 512], F32, tag="ps")
                nc.tensor.matmul(ps[:, 0:iext], P0[half, :], src[0][half, cs],
                                 start=True, stop=False)
                nc.tensor.matmul(ps[:, 0:iext], P1[half, :], src[1][half, cs],
                                 start=False, stop=True)
                t = consts.tile([128, iext], BF, tag=f"GTc{jc}")
                nc.vector.scalar_tensor_tensor(
                    t[:], ps[:, 0:iext], ALPHA / 2.0, trimask[:, 0:iext],
                    op0=ALU.mult, op1=ALU.add)
                eg = consts.tile([128, iext], BF, tag=f"expG{jc}")
                nc.scalar.activation(eg[:], t[:], AF.Exp)
                GTc[jc] = eg

            # interleave so the GTc exps sit mid-stream in the ACT queue:
            # early enough to unblock e2, late enough not to stall the
            # eRaw exps that free the scores psum tiles.
            for hp in range(2):
                for jc in (2, 3, 0, 1):
                    scores_exp(hp, jc)
            make_gtc(2)
            make_gtc(3)
            for jc in (2, 3, 0, 1):
                scores_exp(2, jc)
            make_gtc(0)
            make_gtc(1)
            for jc in (2, 3, 0, 1):
                scores_exp(3, jc)

            proj_v(0)
            proj_v(1)

            # ---------- HAM warm-keepers through the exchange hole --------
            # all pre-exchange PE work drains before the AllGather lands;
            # ~3.4us of PE idle re-throttles the clock to 1.2GHz and the
            # whole tail runs cold.  Burn ~14us of dummy MMs to hold
            # 2.4GHz; reading the LAST eRaw tile pins them to the end of
            # the pre-exchange phase (else the static scheduler hoists
            # them into early DMA-wait gaps where they are useless).
            for dmy in range(46):
                psd = pps.tile([128, 512], F32, tag="ps")
                nc.tensor.matmul(psd[:], eR[3][1][0:128, 0:128],
                                 eR[3][1][0:128, 128:640],
                                 start=True, stop=True)

            # ---------- e2 + row sums ----------
            rs = prs.tile([128, 512], F32, tag="rs")
            ot = [None] * 4
            eT = [[None] * 4 for _ in range(4)]

            def rowsum(hp, jc, start_ok):
                # every MM writes partitions 0..7 (row h = sum, others +0) so
                # a single accumulation group spans all heads
                iext = L - 128 * jc
                for sub in range(2):
                    h = 2 * hp + sub
                    first = start_ok and jc == 2 and sub == 0
                    last = hp == 3 and jc == 1 and sub == 1
                    if jc == 0:
                        # split at the 256-col boundary: left half is fresh
                        # (overwrite-by-bit), right half accumulates
                        for lo, hi in ((0, 256), (256, 512)):
                            nc.tensor.matmul(
                                rs[0:8, lo:hi],
                                ones8[:, 8 * h:8 * h + 8],
                                eT[hp][jc][:, sub * iext + lo:sub * iext + hi],
                                start=False, stop=False)
                    else:
                        nc.tensor.matmul(
                            rs[0:8, 128 * jc:L],
                            ones8[:, 8 * h:8 * h + 8],
                            eT[hp][jc][:, sub * iext:(sub + 1) * iext],
                            start=first, stop=last)

            # GpSimd elementwise is ~2.2x slower than DVE 1x: give it ~30%
            # of the e2 columns so the two engines finish together.
            def emit_e2(hp):
                for jc in (2, 3, 0, 1):
                    iext = L - 128 * jc
                    e2 = etp.tile([128, 2 * iext], BF, tag="eT", bufs=16)
                    for sub in range(2):
                        eng = nc.gpsimd if (jc != 0 and sub == 1) \
                            else nc.vector
                        eng.tensor_mul(
                            e2[:, sub * iext:(sub + 1) * iext],
                            eR[hp][jc][:, sub * iext:(sub + 1) * iext],
                            GTc[jc][:])
                    eT[hp][jc] = e2

            # ---------- attention ----------
            def attention(hp):
                otp = psc.tile([128, 512], F32, tag="sc", bufs=2)
                for sub in range(2):
                    h = 2 * hp + sub
                    po = 64 * sub
                    for jc in (2, 3, 0, 1):
                        iext = L - 128 * jc
                        if jc == 0:
                            for lo, hi in ((0, 256), (256, 512)):
                                nc.tensor.matmul(
                                    otp[po:po + 64, lo:hi],
                                    v[jc][:, h * HD:(h + 1) * HD],
                                    eT[hp][jc][:, sub * iext + lo:sub * iext + hi],
                                    start=False, stop=False,
                                    tile_position=(0, po))
                        else:
                            nc.tensor.matmul(
                                otp[po:po + 64, 128 * jc:L],
                                v[jc][:, h * HD:(h + 1) * HD],
                                eT[hp][jc][:, sub * iext:(sub + 1) * iext],
                                start=(jc == 2), stop=(jc == 1),
                                tile_position=(0, po))
                return otp

            # interleave per head-pair: e2 -> its rowsums -> its attn MMs,
            # so otp work starts as soon as that hp's e2 lands instead of
            # queueing behind all four hps' rowsums.
            otps = [None] * 4
            for hp in range(4):
                emit_e2(hp)
                for jc in (2, 3, 0, 1):
                    rowsum(hp, jc, start_ok=(hp == 0))
                otps[hp] = attention(hp)

            # bridge the recip->ips PE idle window (it would re-throttle
            # the clock right before the output projection)
            for dmy in range(8):
                psd = pps.tile([128, 256], F32, tag="ps")
                nc.tensor.matmul(psd[:], eT[3][1][0:128, 0:128],
                                 eT[3][1][0:128, 128:384],
                                 start=True, stop=True)

            # softmax scale: fast approx 1/x on DVE, cast bf16 on ACT, then
            # a constant matmul broadcasts row h to the head layout.
            invf = work.tile([8, 512], F32, tag="invf")
            nc.vector.reciprocal_approx_fast(out=invf[:], in_=rs[0:8, :])
            inv = consts.tile([8, 512], BF, tag="inv")
            nc.scalar.copy(inv[:], invf[:])

            def att_epilogue(hp):
                ips = pps.tile([128, 512], F32, tag="ps")
                nc.tensor.matmul(ips[:], sel8[:, hp * 128:(hp + 1) * 128],
                                 inv[:], start=True, stop=True)
                otraw = work.tile([128, 512], BF, tag="otraw")
                nc.scalar.copy(otraw[:], otps[hp][:, 0:512])
                t = consts.tile([128, 512], BF, tag=f"ot{hp}")
                nc.vector.scalar_tensor_tensor(t[:], otraw[:], 1.0, ips[:],
                                               op0=ALU.mult, op1=ALU.mult)
                ot[hp] = t

            for hp in range(4):
                att_epilogue(hp)

            # ---------- output projection ----------
            # alternate psum pools (pps + the now-idle sc slots) so chains
            # are MM-paced, not cast-paced; casts alternate DVE/ACT and the
            # output DMA is split in halves across all three rings.
            dma_engs = [nc.sync, nc.gpsimd, nc.scalar]

            def out_proj(ic, nn):
                n = 2 * ic + nn
                if n % 2 == 0:
                    ps = pps.tile([128, 512], F32, tag="ps")
                else:
                    ps = psc.tile([128, 512], F32, tag="sc", bufs=2)
                for dc in range(4):
                    nc.tensor.matmul(ps[:], ot[dc][:, ic * 128:(ic + 1) * 128],
                                     wo[dc][:, nn * 512:(nn + 1) * 512],
                                     start=(dc == 0), stop=(dc == 3))
                osb = work.tile([128, 512], BF, tag="osb", bufs=3)
                if n % 2:
                    nc.scalar.copy(osb[:], ps[:])
                else:
                    nc.vector.tensor_copy(osb[:], ps[:])
                for hh in range(2):
                    dma_engs[(2 * n + hh) % 3].dma_start(
                        out=out_d[ic * 128:(ic + 1) * 128,
                                  nn * 512 + hh * 256:nn * 512 + (hh + 1) * 256],
                        in_=osb[:, hh * 256:(hh + 1) * 256])

            for ic in range(4):
                for nn in range(2):
                    out_proj(ic, nn)

            ctx_att.__exit__(None, None, None)
            pgr_ctx.__exit__(None, None, None)

    nc.compile()
    return nc


def _host_inputs(x, Wq, Wk, Wv, Wo, Wc, We, W1c, W1e, b1, W2, b2):
    """Per-core input dicts (host-side shard/cast/pack)."""
    x = _f32(np.asarray(x))
    wq_s = _f32(np.asarray(Wq) / math.sqrt(HD))
    wk = _f32(np.asarray(Wk))
    wv = _f32(np.asarray(Wv))
    wo = _f32(np.asarray(Wo))
    wc1 = _f32(np.asarray(Wc) @ np.asarray(W1c))      # (D, CD)
    we1 = _f32(np.asarray(We) @ np.asarray(W1e))
    wc1c = wc1.reshape(8, 128, CD).transpose(1, 0, 2)          # (128, 8, CD)
    wc1x4 = np.tile(wc1c[:, :, None, :], (1, 1, 4, 1)).reshape(128, 8 * 128)
    we1r = we1.reshape(8, 128, CD).transpose(1, 0, 2).reshape(128, 8 * CD)
    b1x4 = _f32(np.tile(np.asarray(b1).reshape(1, CD), (4, 1)).reshape(128, 1))
    b2h = _f32(np.full((128, 1), 0.5 * float(np.asarray(b2).reshape(-1)[0])))
    w2 = _f32(np.asarray(W2))

    # w2t[p=u*32+c, t*32 + m'] = W2[c] if m' == 4*(t%8)+u else 0
    # (out partition group 32*(t//8) selected by tile_position at emit time)
    w2t = np.zeros((32, 128, 32), np.float32)
    for t in range(32):
        for u in range(4):
            w2t[t, u * CD:(u + 1) * CD, 4 * (t % 8) + u] = w2
    w2t = w2t.transpose(1, 0, 2).reshape(128, 32 * 32)

    # P_r[4k+u, (8k + 4r + u) % 128] = 1  (fp8 operand for the gx permutes)
    P = np.zeros((2, 128, 128), np.float32)
    for r in range(2):
        for k in range(32):
            for u in range(4):
                P[r, 4 * k + u, (8 * k + 4 * r + u) % 128] = 1.0
    # trimask[m, x] = 0 iff x >= m (i >= j within any j-chunk), else -inf
    trimask = np.where(np.arange(512)[None, :] >= np.arange(128)[:, None],
                       0.0, NEG).astype(np.float32)
    ones8 = np.zeros((128, 64), np.float32)
    for h in range(8):
        ones8[:, 8 * h + h] = 1.0
    sel8 = np.zeros((128, 512), np.float32)
    for hp in range(4):
        sel8[2 * hp, hp * 128:hp * 128 + 64] = 1.0
        sel8[2 * hp + 1, hp * 128 + 64:hp * 128 + 128] = 1.0
    bootc = np.concatenate([wc1x4, we1r], axis=1)

    def hpack(w, cols):  # (1024, cols) -> (128, 8*cols) m-chunk-major
        return w.reshape(8, 128, cols).transpose(1, 0, 2).reshape(128, 8 * cols)

    def hpack_dc(w):  # (1024, 512) -> (128, 4dc*8mc*128) dc-major
        return (w.reshape(8, 128, 4, 128).transpose(1, 2, 0, 3)
                .reshape(128, 4096))

    in_maps = []
    for core in range(N_CORES):
        b, g = core // 2, core % 2
        hd0 = g * DPC                                  # head-group d offset
        xTb = np.ascontiguousarray(x[b].T)             # (D, L)
        # hej columns: own A rows (j = 8k+4g+u, k<32), then B rows for both
        # rank parities (j = 8k+4r+u, k in 32..64, r = 0 then 1)
        jcols = np.array([8 * kk + 4 * g + u
                          for u in range(4) for kk in range(64)])
        xjeb = np.ascontiguousarray(xTb[:, jcols])     # (D, 384)
        cpk = np.concatenate([w2t, ones8, sel8], axis=1)
        assert cpk.shape[1] == CPK8
        in_maps.append({
            "boot": _f8(bootc),
            "xta": _bf(hpack(xTb, L)),
            "xje": _f8(hpack(xjeb, 256)),
            "cpk": _f8(cpk),
            "tri": _bf(trimask),
            "p8": np.ascontiguousarray(
                np.concatenate([P[0], P[1]], axis=1)
                .astype(ml_dtypes.float8_e4m3)),
            "wka": _bf(hpack_dc(wk[:, hd0:hd0 + DPC])),
            "wqa": _bf(hpack_dc(wq_s[:, hd0:hd0 + DPC])),
            "wva": _bf(hpack(wv[:, hd0:hd0 + DPC], DPC)),
            "woa": _bf(np.ascontiguousarray(
                wo[hd0:hd0 + DPC].reshape(4, 128, D)
                .transpose(1, 0, 2).reshape(128, 4 * D))),
            "b1x4": b1x4, "b2h": b2h,
        })
    return in_maps


def run(inputs: dict, trace: bool = False):
    """Build, run on 8 cores, return (full_output, BassKernelResults)."""
    nc = build_program()
    in_maps = _host_inputs(**inputs)
    res = run_bass_kernel_spmd(nc, in_maps, core_ids=list(range(N_CORES)),
                               trace=trace)
    out = np.zeros((B, L, D), np.float32)
    for b in range(B):
        out[b] = (res.results[2 * b]["out"].astype(np.float32)
                  + res.results[2 * b + 1]["out"].astype(np.float32))
    return out, res


def kernel(**inputs) -> np.ndarray:
    out, _ = run(inputs, trace=False)
    return out
